# revision 44
# baseline (speedup 1.0000x reference)
"""2-layer GAT on 8 TRN2 NeuronCores.

Strategy (per-edge random access is unavailable in this environment — the
extended dma_gather ucode crashes and indirect DMA runs ~1.4us/128 rows — so
all device memory traffic is sequential streams; per-edge irregularity is
encoded host-side from edge_index into streams, and segment softmax/reduce
run on the PE via one-hot staircase matmuls):

  Launch A (1-D node shard, transposed): h|as|ad = W1ext^T @ x^T with
    W1ext = [W1 | W1·blockdiag(a_src) | W1·blockdiag(a_dst)] built on
    device; bf16 streams; outputs TT=[80, nodes] bf16 tables.
  Host: permute tables into dst-sorted per-edge streams (layout only),
    including the one-hot segment matrices as fp8 (exact 0/1).
  Launch B (1-D dst shard): p = exp(prelu(as+ad)) widened on ACT;
    wall = [p*h | p] via 2x-mode bf16 TT; per 128-edge chunk matmul
    (fp8 one-hot stationary x bf16 moving) accumulates S|z into PSUM;
    batched epilogue: out1 = S/z + b1, elu, tables for layer 2.
  Host: permute layer-2 tables into streams.
  Launch C: same machinery with H=1, C=7; log_softmax; output shard.

Numerics: segment-softmax max-subtraction is skipped (logit scale here is
~|e|<2 so exp is safe); softmax is alpha = p / sum(p), identical math.
"""
import numpy as np
import ml_dtypes

import concourse.bass as bass
import concourse.mybir as mybir
import concourse.tile as tile
from concourse import bacc
from concourse.masks import make_identity
from concourse.bass_utils import run_bass_kernel_spmd

F32 = mybir.dt.float32
BF16 = mybir.dt.bfloat16
F8 = mybir.dt.float8e4
AF = mybir.ActivationFunctionType
ALU = mybir.AluOpType

N = 100000
E = 1600000
F_IN = 512
H = 8
D = 8
HD = 64
C = 7
NEG = 0.2
NCORES = 8
NSHARD = N // NCORES          # 12500
P = 128
# launch A geometry (512-node tiles, transposed output)
TA = 512
NTA = -(-NSHARD // TA)         # 25
NPAD_A = NTA * TA              # 12800
# launch B/C geometry (128-node tiles)
NTILE = (NSHARD + P - 1) // P  # 98
NPAD = NTILE * P               # 12544
WIN = 64
NSLOT = NTILE * 2              # 196
KSUP_B = 116                   # chunks per superchunk, layer 1
KSUP_C = 232                   # chunks per superchunk, layer 2
KPAD = 232                     # kt padded to this (multiple of both)
R1 = 72                        # [p*h(64) | p(8)]
R2 = 8                         # [p*h2(7) | p(1)]
EG = 7                         # epilogue batch (tiles)
STAGE_G = 14                   # tiles per output staging flush

FP8_ONE = 0x38                 # fp8_e4m3 bit pattern of 1.0


# ---------------------------------------------------------------- host prep

def build_structure(edge_index, win):
    """Edge_index-derived structure for one window size. Returns chunk
    metadata, per-core edge placements, and per-core fp8 one-hot streams."""
    src = np.concatenate([edge_index[0], np.arange(N, dtype=np.int64)]).astype(np.int64)
    dst = np.concatenate([edge_index[1], np.arange(N, dtype=np.int64)]).astype(np.int64)
    nw = P // win                 # windows per 128-node tile
    nslot = NTILE * nw
    shift = win.bit_length() - 1

    cores = []
    counts = np.zeros((NCORES, nslot), np.int64)
    for k in range(NCORES):
        lo = k * NSHARD
        sel = (dst >= lo) & (dst < lo + NSHARD)
        s_k = src[sel].astype(np.int32)
        d_k = (dst[sel] - lo).astype(np.int32)
        order = np.argsort(d_k, kind="stable")
        s_k, d_k = s_k[order], d_k[order]
        slot = d_k >> shift
        counts[k] = np.bincount(slot, minlength=nslot)
        cores.append((s_k, d_k, slot))

    cs = np.maximum(1, -(-counts.max(axis=0) // P))  # chunks per slot (shared)
    kt_real = int(cs.sum())
    kt = -(-kt_real // KPAD) * KPAD               # pad to superchunk multiple

    # shared chunk metadata
    chunk_tile = np.empty(kt, np.int32)
    chunk_b = np.empty(kt, np.int32)
    chunk_start = np.zeros(kt, bool)
    chunk_stop = np.zeros(kt, bool)
    chunk_epi = np.full(kt, -1, np.int32)  # tile to epilogue after this chunk
    slot_off = np.zeros(nslot + 1, np.int64)
    c = 0
    for s in range(nslot):
        t, w = s // nw, s % nw
        slot_off[s] = c
        for j in range(int(cs[s])):
            chunk_tile[c] = t
            chunk_b[c] = w * win
            chunk_start[c] = j == 0
            chunk_stop[c] = j == int(cs[s]) - 1
            c += 1
        if w == nw - 1:
            chunk_epi[c - 1] = t
    slot_off[nslot] = c
    assert c == kt_real
    chunk_tile[kt_real:] = -1  # trash chunks

    # per-core edge placement: position of edge i in the padded stream
    placements = []
    mt_streams = []
    for k in range(NCORES):
        s_k, d_k, slot = cores[k]
        cnt = np.bincount(slot, minlength=nslot)
        starts = slot_off[:-1] * P
        idx_in_slot = np.arange(len(s_k)) - np.concatenate(
            [[0], np.cumsum(cnt)])[slot]
        pos = (starts[slot] + idx_in_slot).astype(np.int64)
        placements.append((s_k, d_k, pos))
        # fp8 one-hot stream: row e of chunk c has 1.0 at col (d_k % win)
        mtb = np.zeros((kt * P, win), np.uint8)
        mtb[pos, d_k & (win - 1)] = FP8_ONE
        mt_streams.append(
            to_stream(mtb, kt, win).view(ml_dtypes.float8_e4m3fn))

    meta = dict(kt=kt, kt_real=kt_real, win=win, chunk_tile=chunk_tile,
                chunk_b=chunk_b, chunk_start=chunk_start,
                chunk_stop=chunk_stop, chunk_epi=chunk_epi)
    return meta, placements, mt_streams


def to_stream(arr, kt, w):
    """[kt*128, w] -> [128, kt*w] (edge j = c*128 + p -> [p, c*w:(c+1)*w])."""
    return np.ascontiguousarray(
        arr.reshape(kt, P, w).transpose(1, 0, 2).reshape(P, kt * w))


def make_streams(placements, kt, tab_h, tab_s, tab_ad, wh, ws):
    """Gathered per-edge streams from node tables (host layout op).
    Tables arrive in bf16."""
    ghs, sss, ads = [], [], []
    bf = ml_dtypes.bfloat16
    for k in range(NCORES):
        s_k, d_k, pos = placements[k]
        lo = k * NSHARD
        gh = np.zeros((kt * P, wh), bf)
        gh[pos] = tab_h[s_k]
        ss = np.full((kt * P, ws), -1e9, bf)
        ss[pos] = tab_s[s_k]
        ad = np.zeros((kt * P, ws), bf)
        ad[pos] = tab_ad[lo + d_k]
        ghs.append(to_stream(gh, kt, wh))
        sss.append(to_stream(ss, kt, ws))
        ads.append(to_stream(ad, kt, ws))
    return ghs, sss, ads


# ---------------------------------------------------------------- launch A

def build_A(reps=1):
    nc = bacc.Bacc("TRN2", target_bir_lowering=False)
    xt_in = nc.dram_tensor("XT", [P, 4 * NPAD_A], BF16, kind="ExternalInput")
    w1_in = nc.dram_tensor("W1", [F_IN, HD], F32, kind="ExternalInput")
    abd_in = nc.dram_tensor("ABD", [HD, 16], F32, kind="ExternalInput")
    tt_out = nc.dram_tensor("TT", [80, NPAD_A], BF16, kind="ExternalOutput")

    with tile.TileContext(nc) as tc:
        with (
            tc.tile_pool(name="const", bufs=1) as cpool,
            tc.tile_pool(name="xt", bufs=2) as xpool,
            tc.tile_pool(name="out", bufs=4) as opool,
            tc.tile_pool(name="tmp", bufs=2) as tpool,
            tc.tile_pool(name="ps", bufs=4, space="PSUM") as ppool,
            tc.tile_pool(name="pst", bufs=2, space="PSUM") as ptpool,
        ):
            ident = cpool.tile([P, P], F32)
            make_identity(nc, ident[:])
            w1f = cpool.tile([P, 4 * HD], F32)
            nc.sync.dma_start(
                out=w1f[:].rearrange("k (c n) -> k c n", c=4),
                in_=w1_in[:, :].rearrange("(c k) n -> k c n", k=P))
            abd = cpool.tile([HD, 16], F32)
            nc.sync.dma_start(out=abd[:], in_=abd_in[:, :])

            # W1ext = [W1 | W1@blockdiag(a_src) | W1@blockdiag(a_dst)]
            w1ext = cpool.tile([P, 4 * 80], BF16)
            for c in range(4):
                w1c = w1f[:, c * HD:(c + 1) * HD]
                trp = ptpool.tile([HD, P], F32, tag="trp")
                nc.tensor.transpose(trp[:], w1c, ident[:])
                trs = tpool.tile([HD, P], F32, tag="trs")
                nc.vector.tensor_copy(out=trs[:], in_=trp[:])
                pse = ptpool.tile([P, 16], F32, tag="pse")
                nc.tensor.matmul(pse[:], trs[:], abd[:], start=True, stop=True)
                nc.vector.tensor_copy(out=w1ext[:, c * 80:c * 80 + HD], in_=w1c)
                nc.vector.tensor_copy(
                    out=w1ext[:, c * 80 + HD:(c + 1) * 80], in_=pse[:])

            xt_d = xt_in[:, :].rearrange("k (c n) -> k c n", c=4)
            TPD = 5  # tiles per DMA
            for rep in range(reps):
                for t in range(NTA):
                    if t % TPD == 0:
                        nt = min(TPD, NTA - t)
                        xbuf = xpool.tile([P, 4 * TPD * TA], BF16, tag="xbuf")
                        xv = xbuf[:].rearrange("k (c n) -> k c n", c=4)
                        nc.sync.dma_start(
                            out=xv[:, :, 0:nt * TA],
                            in_=xt_d[:, :, t * TA:(t + nt) * TA])
                    lc = (t % TPD) * TA
                    ps = ppool.tile([80, TA], F32)
                    for c in range(4):
                        nc.tensor.matmul(
                            ps[:], w1ext[:, c * 80:(c + 1) * 80],
                            xv[:, c, lc:lc + TA],
                            start=(c == 0), stop=(c == 3))
                    ob = opool.tile([80, TA], BF16, tag="ob")
                    nc.vector.tensor_copy(out=ob[:], in_=ps[:])
                    nc.sync.dma_start(
                        out=tt_out[:, t * TA:(t + 1) * TA], in_=ob[:])
    nc.compile()
    return nc


# ---------------------------------------------------------------- launch B

def build_edge_launch_B(meta, layer=1, reps=1):
    """Layer-1 edge launch (known-good HW form): fp8 one-hot stationary,
    ACT-widened p, fp32 per-tile epilogue, (h,d) column order."""
    kt = meta["kt"]
    win = meta["win"]
    KSUP = 64
    nsup = kt // KSUP
    wh, ws, R, nh, dd = HD, H, R1, H, D

    nc = bacc.Bacc("TRN2", target_bir_lowering=False)
    gh_in = nc.dram_tensor("GH", [P, kt * wh], BF16, kind="ExternalInput")
    ss_in = nc.dram_tensor("SS", [P, kt * ws], BF16, kind="ExternalInput")
    ad_in = nc.dram_tensor("AD", [P, kt * ws], BF16, kind="ExternalInput")
    mt_in = nc.dram_tensor("MT", [P, kt * win], F8, kind="ExternalInput")
    b1_in = nc.dram_tensor("B1", [HD], F32, kind="ExternalInput")
    w2_in = nc.dram_tensor("W2", [HD, C], F32, kind="ExternalInput")
    a2s_in = nc.dram_tensor("A2S", [C], F32, kind="ExternalInput")
    a2d_in = nc.dram_tensor("A2D", [C], F32, kind="ExternalInput")
    t2_out = nc.dram_tensor("T2", [NPAD, 9], BF16, kind="ExternalOutput")
    stage_cols = 9

    with tile.TileContext(nc) as tc:
        with (
            tc.tile_pool(name="const", bufs=1) as cpool,
            tc.tile_pool(name="stream", bufs=3) as dpool,
            tc.tile_pool(name="work", bufs=3) as wpool,
            tc.tile_pool(name="epi", bufs=2) as epool,
            tc.tile_pool(name="stage", bufs=2) as spool,
            tc.tile_pool(name="ps", bufs=2, space="PSUM") as ppool,
            tc.tile_pool(name="trashp", bufs=1, space="PSUM") as trpool,
            tc.tile_pool(name="pst", bufs=2, space="PSUM") as ptpool,
        ):
            ident = cpool.tile([P, P], F32)
            make_identity(nc, ident[:])
            b1r = cpool.tile([P, HD], F32)
            nc.sync.dma_start(out=b1r[:],
                              in_=b1_in[None, :].to_broadcast([P, HD]))
            w2 = cpool.tile([P, C], F32)
            nc.sync.dma_start(out=w2[0:HD, :], in_=w2_in[:, :])
            a2sr = cpool.tile([P, C], F32)
            nc.sync.dma_start(out=a2sr[0:HD, :],
                              in_=a2s_in[None, :].to_broadcast([HD, C]))
            a2dr = cpool.tile([P, C], F32)
            nc.sync.dma_start(out=a2dr[0:HD, :],
                              in_=a2d_in[None, :].to_broadcast([HD, C]))
            # W2cat = [W2 | W2 @ a2s^T | W2 @ a2d^T]  ([64, 9])
            w2cat = cpool.tile([P, 9], F32)
            nc.vector.tensor_copy(out=w2cat[0:HD, 0:C], in_=w2[0:HD, :])
            tmpw = cpool.tile([P, C], F32)
            nc.vector.tensor_tensor(out=tmpw[0:HD, :], in0=w2[0:HD, :],
                                    in1=a2sr[0:HD, :], op=ALU.mult)
            nc.vector.reduce_sum(out=w2cat[0:HD, C:C + 1], in_=tmpw[0:HD, :],
                                 axis=mybir.AxisListType.X)
            nc.vector.tensor_tensor(out=tmpw[0:HD, :], in0=w2[0:HD, :],
                                    in1=a2dr[0:HD, :], op=ALU.mult)
            nc.vector.reduce_sum(out=w2cat[0:HD, C + 1:C + 2],
                                 in_=tmpw[0:HD, :],
                                 axis=mybir.AxisListType.X)

            trash = trpool.tile([P, R], F32, tag="trash")
            trash_used = [False]

            chunk_tile = meta["chunk_tile"]
            chunk_b = meta["chunk_b"]
            chunk_start = meta["chunk_start"]
            chunk_stop = meta["chunk_stop"]
            chunk_epi = meta["chunk_epi"]

            psum_by_tile = {}
            stage_tile = [None]
            stage = {"g0": 0}

            def flush_stage(t_last):
                g0 = stage["g0"]
                ng = t_last - g0 + 1
                st = stage_tile[0]
                nc.sync.dma_start(
                    out=t2_out[g0 * P:(g0 + ng) * P, :].rearrange(
                        "(g p) c -> p g c", p=P),
                    in_=st[:, 0:ng * stage_cols].rearrange(
                        "p (g c) -> p g c", g=ng))
                stage_tile[0] = None

            def epilogue(t, ps):
                zrec = epool.tile([P, nh], F32, tag="zrec")
                nc.vector.reciprocal(out=zrec[:], in_=ps[:, wh:wh + nh])
                o1 = epool.tile([P, wh], F32, tag="o1")
                nc.vector.tensor_tensor(
                    out=o1[:].rearrange("p (h d) -> p h d", h=nh),
                    in0=ps[:, 0:wh].rearrange("p (h d) -> p h d", h=nh),
                    in1=zrec[:, :, None].to_broadcast([P, nh, dd]),
                    op=ALU.mult)
                if stage_tile[0] is None:
                    stage_tile[0] = spool.tile(
                        [P, STAGE_G * stage_cols], BF16, tag="stage",
                        name=f"stage{t}")
                    stage["g0"] = t
                g = t - stage["g0"]
                # h = elu(o1 + b1); T2 = [h@W2 | h@W2a2s | h@W2a2d]
                nc.vector.tensor_tensor(out=o1[:], in0=o1[:], in1=b1r[:],
                                        op=ALU.add)
                mn = epool.tile([P, wh], F32, tag="mn")
                nc.vector.tensor_scalar_min(out=mn[:], in0=o1[:], scalar1=0.0)
                nc.scalar.activation(mn[:], mn[:], AF.Exp)
                mx = epool.tile([P, wh], F32, tag="mx")
                nc.vector.tensor_scalar_max(out=mx[:], in0=o1[:], scalar1=0.0)
                nc.vector.tensor_tensor(out=o1[:], in0=mx[:], in1=mn[:],
                                        op=ALU.add)
                nc.vector.tensor_scalar_add(out=o1[:], in0=o1[:], scalar1=-1.0)
                trp = ptpool.tile([HD, P], F32, tag="trp")
                nc.tensor.transpose(trp[:], o1[:], ident[:])
                trs = epool.tile([P, P], F32, tag="trs")
                nc.vector.tensor_copy(out=trs[0:HD, :], in_=trp[:])
                h2p = ptpool.tile([P, 9], F32, tag="h2p")
                nc.tensor.matmul(h2p[:], trs[0:HD, :], w2cat[0:HD, :],
                                 start=True, stop=True)
                nc.vector.tensor_copy(
                    out=stage_tile[0][:, g * 9:(g + 1) * 9], in_=h2p[:])
                if g == STAGE_G - 1 or t == NTILE - 1:
                    flush_stage(t)

            for rep in range(reps):
              for sc in range(nsup):
                  gh = dpool.tile([P, KSUP * wh], BF16, tag="gh")
                  nc.sync.dma_start(out=gh[:],
                                    in_=gh_in[:, sc * KSUP * wh:(sc + 1) * KSUP * wh])
                  ssb = dpool.tile([P, KSUP * ws], BF16, tag="ssb")
                  nc.sync.dma_start(out=ssb[:],
                                    in_=ss_in[:, sc * KSUP * ws:(sc + 1) * KSUP * ws])
                  adb = dpool.tile([P, KSUP * ws], BF16, tag="adb")
                  nc.sync.dma_start(out=adb[:],
                                    in_=ad_in[:, sc * KSUP * ws:(sc + 1) * KSUP * ws])
                  mtb = dpool.tile([P, KSUP * win], F8, tag="mtb")
                  nc.sync.dma_start(out=mtb[:],
                                    in_=mt_in[:, sc * KSUP * win:(sc + 1) * KSUP * win])

                  e8 = wpool.tile([P, KSUP * ws], BF16, tag="e8")
                  nc.vector.tensor_tensor(out=e8[:], in0=ssb[:], in1=adb[:],
                                          op=ALU.add)
                  nc.scalar.activation(e8[:], e8[:], AF.Prelu, alpha=NEG)
                  wall = wpool.tile([P, KSUP * R], BF16, tag="wall")
                  wall_v = wall[:].rearrange("p (k r) -> p k r", k=KSUP)
                  nc.scalar.activation(
                      wall_v[:, :, wh:R],
                      e8[:].rearrange("p (k s) -> p k s", k=KSUP), AF.Exp)
                  pw = wpool.tile([P, KSUP * wh], BF16, tag="pw")
                  nc.scalar.activation(
                      pw[:].rearrange("p (k h d) -> p k h d", k=KSUP, h=nh),
                      e8[:].rearrange("p (k h) -> p k h", k=KSUP)
                      [:, :, :, None].to_broadcast([P, KSUP, nh, dd]),
                      AF.Exp)
                  nc.vector.tensor_tensor(
                      out=wall_v[:, :, 0:wh],
                      in0=gh[:].rearrange("p (k c) -> p k c", k=KSUP),
                      in1=pw[:].rearrange("p (k c) -> p k c", k=KSUP),
                      op=ALU.mult)

                  for j in range(KSUP):
                      cidx = sc * KSUP + j
                      t = int(chunk_tile[cidx])
                      lhs = mtb[:, j * win:(j + 1) * win]
                      rhs = wall[:, j * R:(j + 1) * R]
                      if t < 0:
                          nc.tensor.matmul(trash[0:win, :], lhs, rhs,
                                           start=not trash_used[0], stop=False,
                                           skip_group_check=True)
                          trash_used[0] = True
                          continue
                      b = int(chunk_b[cidx])
                      if chunk_start[cidx] and b == 0:
                          psum_by_tile[t] = ppool.tile([P, R], F32, tag="acc",
                                                       name=f"acc{t}")
                      ps = psum_by_tile[t]
                      nc.tensor.matmul(
                          ps[b:b + win, :], lhs, rhs,
                          start=bool(chunk_start[cidx]),
                          stop=bool(chunk_stop[cidx]),
                          skip_group_check=True)
                      te = int(chunk_epi[cidx])
                      if te >= 0:
                          epilogue(te, psum_by_tile.pop(te))
    nc.compile()
    return nc


# ---------------------------------------------------------------- launch B/C

def build_edge_launch(meta, layer, reps=1):
    """layer 1: R=72 (8 heads), outputs T2 tables (bf16).
    layer 2: R=8 (1 head), outputs log-softmax shard (f32)."""
    kt = meta["kt"]
    win = meta["win"]
    KSUP = KSUP_B if layer == 1 else KSUP_C
    KS = 58 if layer == 1 else 116          # chunks per work slice
    nsup = kt // KSUP
    wh = HD if layer == 1 else C            # gathered h width
    ws = H if layer == 1 else 1             # as/ad width
    R = R1 if layer == 1 else R2
    nh = ws
    dd = D if layer == 1 else C

    nc = bacc.Bacc("TRN2", target_bir_lowering=False)
    gh_in = nc.dram_tensor("GH", [P, kt * wh], BF16, kind="ExternalInput")
    ss_in = nc.dram_tensor("SS", [P, kt * ws], BF16, kind="ExternalInput")
    ad_in = nc.dram_tensor("AD", [P, kt * ws], BF16, kind="ExternalInput")
    mt_in = nc.dram_tensor("MT", [P, kt * win], F8, kind="ExternalInput")
    if layer == 1:
        b1_in = nc.dram_tensor("B1", [HD], F32, kind="ExternalInput")
        w2_in = nc.dram_tensor("W2", [HD, C], F32, kind="ExternalInput")
        a2s_in = nc.dram_tensor("A2S", [C], F32, kind="ExternalInput")
        a2d_in = nc.dram_tensor("A2D", [C], F32, kind="ExternalInput")
        t2_out = nc.dram_tensor("T2", [NPAD, 9], BF16, kind="ExternalOutput")
        stage_cols = 9
    else:
        b2_in = nc.dram_tensor("B2", [C], F32, kind="ExternalInput")
        out_out = nc.dram_tensor("OUT", [NPAD, C], F32, kind="ExternalOutput")
        stage_cols = C

    with tile.TileContext(nc) as tc:
        with (
            tc.tile_pool(name="const", bufs=1) as cpool,
            tc.tile_pool(name="stream", bufs=3) as dpool,
            tc.tile_pool(name="work", bufs=2) as wpool,
            tc.tile_pool(name="epi", bufs=2) as epool,
            tc.tile_pool(name="bat", bufs=2) as bpool,
            tc.tile_pool(name="stage", bufs=2) as spool,
            tc.tile_pool(name="ps", bufs=2, space="PSUM") as ppool,
            tc.tile_pool(name="trashp", bufs=1, space="PSUM") as trpool,
            tc.tile_pool(name="pst", bufs=2, space="PSUM") as ptpool,
            tc.tile_pool(name="psb", bufs=2, space="PSUM") as pbpool,
        ):
            # pin the one act-function table that covers every function we
            # use, so the compiler's fixpoint pass inserts no reloads
            from concourse.hw_specs import get_activation_tables
            req = {AF.Exp, AF.Prelu} if layer == 1 else {AF.Exp, AF.Prelu, AF.Ln}
            tabs = get_activation_tables(nc.m.arch)
            sid = next(i for i, fns in enumerate(tabs.values()) if req <= fns)
            nc.scalar.add_instruction(mybir.InstLoadActFuncSet(
                name=nc.get_next_instruction_name(), ins=[], outs=[],
                act_func_set_id=sid))

            if layer == 1:
                ident = cpool.tile([P, P], BF16)
                make_identity(nc, ident[:])
                b1r = cpool.tile([P, HD], F32)
                nc.sync.dma_start(out=b1r[:],
                                  in_=b1_in[None, :].to_broadcast([P, HD]))
                # layer-1 feature columns run in (d, h) order (so the DVE
                # p-widening reads contiguous head blocks); permute W2's
                # rows and b1 to match. All tiles are full 128-partition so
                # the pool never packs two of them into one SBUF column.
                w2 = cpool.tile([P, C], F32)
                nc.sync.dma_start(
                    out=w2[0:HD, :].rearrange("(d h) c -> d h c", d=D),
                    in_=w2_in[:, :].rearrange("(h d) c -> d h c", h=H))
                a2sr = cpool.tile([P, C], F32)
                nc.sync.dma_start(out=a2sr[0:HD, :],
                                  in_=a2s_in[None, :].to_broadcast([HD, C]))
                a2dr = cpool.tile([P, C], F32)
                nc.sync.dma_start(out=a2dr[0:HD, :],
                                  in_=a2d_in[None, :].to_broadcast([HD, C]))
                # W2cat = [W2 | W2 @ a2s^T | W2 @ a2d^T]  ([64, 9])
                w2catf = cpool.tile([P, 9], F32)
                nc.vector.tensor_copy(out=w2catf[0:HD, 0:C], in_=w2[0:HD, :])
                tmpw = cpool.tile([P, C], F32)
                nc.vector.tensor_tensor(out=tmpw[0:HD, :], in0=w2[0:HD, :],
                                        in1=a2sr[0:HD, :], op=ALU.mult)
                nc.vector.reduce_sum(out=w2catf[0:HD, C:C + 1],
                                     in_=tmpw[0:HD, :],
                                     axis=mybir.AxisListType.X)
                nc.vector.tensor_tensor(out=tmpw[0:HD, :], in0=w2[0:HD, :],
                                        in1=a2dr[0:HD, :], op=ALU.mult)
                nc.vector.reduce_sum(out=w2catf[0:HD, C + 1:C + 2],
                                     in_=tmpw[0:HD, :],
                                     axis=mybir.AxisListType.X)
                w2cat = cpool.tile([P, 9], BF16)
                nc.vector.tensor_copy(out=w2cat[0:HD, :], in_=w2catf[0:HD, :])
            else:
                b2r = cpool.tile([P, C], F32)
                nc.sync.dma_start(out=b2r[:],
                                  in_=b2_in[None, :].to_broadcast([P, C]))
            if layer == 1:
                b1b = cpool.tile([P, HD], BF16)
                nc.vector.tensor_copy(
                    out=b1b[:].rearrange("p (d h) -> p d h", d=D),
                    in_=b1r[:].rearrange("p (h d) -> p d h", h=H))

            trash = trpool.tile([P, R], F32, tag="trash")
            trash_used = [False]

            chunk_tile = meta["chunk_tile"]
            chunk_b = meta["chunk_b"]
            chunk_start = meta["chunk_start"]
            chunk_stop = meta["chunk_stop"]
            chunk_epi = meta["chunk_epi"]

            psum_by_tile = {}
            stage_tile = [None]
            stage = {"g0": 0}
            batch = {"bb": None, "t0": 0, "n": 0}

            def flush_stage(t_last):
                g0 = stage["g0"]
                ng = t_last - g0 + 1
                st = stage_tile[0]
                out_t = t2_out if layer == 1 else out_out
                nc.sync.dma_start(
                    out=out_t[g0 * P:(g0 + ng) * P, :].rearrange(
                        "(g p) c -> p g c", p=P),
                    in_=st[:, 0:ng * stage_cols].rearrange(
                        "p (g c) -> p g c", g=ng))
                stage_tile[0] = None

            def stage_slot(t):
                """Column slice of the staging tile for tile t."""
                if stage_tile[0] is None:
                    stage_tile[0] = spool.tile(
                        [P, STAGE_G * stage_cols],
                        BF16 if layer == 1 else F32,
                        tag="stage", name=f"stage{t}")
                    stage["g0"] = t
                g = t - stage["g0"]
                return stage_tile[0][:, g * stage_cols:(g + 1) * stage_cols]

            def flush_batch_l1(t_last):
                """Batched elu + layer-2 table build for tiles t0..t_last."""
                ng = batch["n"]
                bb = batch["bb"]
                W = ng * HD
                nc.vector.tensor_tensor(
                    out=bb[:, 0:W].rearrange("p (g c) -> p g c", g=ng),
                    in0=bb[:, 0:W].rearrange("p (g c) -> p g c", g=ng),
                    in1=b1b[:, None, :].to_broadcast([P, ng, HD]),
                    op=ALU.add)
                mn = bpool.tile([P, EG * HD], BF16, tag="mn")
                nc.vector.tensor_scalar(out=mn[:, 0:W], in0=bb[:, 0:W],
                                        scalar1=0.0, scalar2=None,
                                        op0=ALU.min)
                nc.scalar.activation(mn[:, 0:W], mn[:, 0:W], AF.Exp)
                nc.vector.tensor_scalar(out=bb[:, 0:W], in0=bb[:, 0:W],
                                        scalar1=0.0, scalar2=-1.0,
                                        op0=ALU.max, op1=ALU.add)
                nc.vector.tensor_tensor(out=bb[:, 0:W], in0=bb[:, 0:W],
                                        in1=mn[:, 0:W], op=ALU.add)
                h2b = pbpool.tile([P, EG * 9], F32, tag="h2b")
                for g in range(ng):
                    t = batch["t0"] + g
                    trp = ptpool.tile([HD, P], BF16, tag="trp")
                    nc.tensor.transpose(
                        trp[:], bb[:, g * HD:(g + 1) * HD], ident[:])
                    trs = epool.tile([P, P], BF16, tag="trs")
                    nc.vector.tensor_copy(out=trs[0:HD, :], in_=trp[:])
                    nc.tensor.matmul(h2b[:, g * 9:(g + 1) * 9], trs[0:HD, :],
                                     w2cat[0:HD, :], start=True, stop=True,
                                     skip_group_check=True)
                for g in range(ng):
                    t = batch["t0"] + g
                    nc.vector.tensor_copy(out=stage_slot(t),
                                          in_=h2b[:, g * 9:(g + 1) * 9])
                    if (t - stage["g0"]) == STAGE_G - 1 or t == NTILE - 1:
                        flush_stage(t)
                batch["bb"] = None
                batch["n"] = 0

            def flush_batch_l2(t_last):
                """Batched log_softmax for tiles t0..t_last."""
                ng = batch["n"]
                bb = batch["bb"]
                W = ng * C
                nc.vector.tensor_tensor(
                    out=bb[:, 0:W].rearrange("p (g c) -> p g c", g=ng),
                    in0=bb[:, 0:W].rearrange("p (g c) -> p g c", g=ng),
                    in1=b2r[:, None, :].to_broadcast([P, ng, C]),
                    op=ALU.add)
                mx = bpool.tile([P, EG], F32, tag="mx")
                nc.vector.reduce_max(
                    out=mx[:, 0:ng],
                    in_=bb[:, 0:W].rearrange("p (g c) -> p g c", g=ng),
                    axis=mybir.AxisListType.X)
                nc.vector.tensor_tensor(
                    out=bb[:, 0:W].rearrange("p (g c) -> p g c", g=ng),
                    in0=bb[:, 0:W].rearrange("p (g c) -> p g c", g=ng),
                    in1=mx[:, 0:ng, None].to_broadcast([P, ng, C]),
                    op=ALU.subtract)
                ee = bpool.tile([P, EG * C], F32, tag="ee")
                nc.scalar.activation(ee[:, 0:W], bb[:, 0:W], AF.Exp)
                z2 = bpool.tile([P, EG], F32, tag="z2")
                nc.vector.reduce_sum(
                    out=z2[:, 0:ng],
                    in_=ee[:, 0:W].rearrange("p (g c) -> p g c", g=ng),
                    axis=mybir.AxisListType.X)
                nc.scalar.activation(z2[:, 0:ng], z2[:, 0:ng], AF.Ln)
                for g in range(ng):
                    t = batch["t0"] + g
                    nc.vector.tensor_tensor(
                        out=stage_slot(t),
                        in0=bb[:, g * C:(g + 1) * C],
                        in1=z2[:, g:g + 1].to_broadcast([P, C]),
                        op=ALU.subtract)
                    if (t - stage["g0"]) == STAGE_G - 1 or t == NTILE - 1:
                        flush_stage(t)
                batch["bb"] = None
                batch["n"] = 0

            def epilogue(t, ps):
                # phase 1: divide by z, move into batch buffer, free psum
                zrec = epool.tile([P, nh], F32, tag="zrec")
                nc.vector.reciprocal(out=zrec[:], in_=ps[:, wh:wh + nh])
                if batch["bb"] is None:
                    batch["bb"] = bpool.tile([P, EG * wh],
                                             BF16 if layer == 1 else F32,
                                             tag="bb", name=f"bb{t}")
                    batch["t0"] = t
                g = batch["n"]
                bb = batch["bb"]
                if layer == 1:  # columns are (d, h)-ordered
                    nc.vector.tensor_tensor(
                        out=bb[:, g * wh:(g + 1) * wh].rearrange(
                            "p (d h) -> p d h", d=dd),
                        in0=ps[:, 0:wh].rearrange("p (d h) -> p d h", d=dd),
                        in1=zrec[:, None, :].to_broadcast([P, dd, nh]),
                        op=ALU.mult)
                else:
                    nc.vector.tensor_tensor(
                        out=bb[:, g * wh:(g + 1) * wh].rearrange(
                            "p (h d) -> p h d", h=nh),
                        in0=ps[:, 0:wh].rearrange("p (h d) -> p h d", h=nh),
                        in1=zrec[:, :, None].to_broadcast([P, nh, dd]),
                        op=ALU.mult)
                batch["n"] += 1
                if batch["n"] == EG or t == NTILE - 1:
                    if layer == 1:
                        flush_batch_l1(t)
                    else:
                        flush_batch_l2(t)

            for rep in range(reps):
              for sc in range(nsup):
                  gh = dpool.tile([P, KSUP * wh], BF16, tag="gh")
                  nc.sync.dma_start(out=gh[:],
                                    in_=gh_in[:, sc * KSUP * wh:(sc + 1) * KSUP * wh])
                  ssb = dpool.tile([P, KSUP * ws], BF16, tag="ssb")
                  nc.sync.dma_start(out=ssb[:],
                                    in_=ss_in[:, sc * KSUP * ws:(sc + 1) * KSUP * ws])
                  adb = dpool.tile([P, KSUP * ws], BF16, tag="adb")
                  nc.sync.dma_start(out=adb[:],
                                    in_=ad_in[:, sc * KSUP * ws:(sc + 1) * KSUP * ws])
                  mtb = dpool.tile([P, KSUP * win], F8, tag="mtb")
                  nc.sync.dma_start(out=mtb[:],
                                    in_=mt_in[:, sc * KSUP * win:(sc + 1) * KSUP * win])

                  # e = prelu(as+ad); p = exp(e); pw = p widened to dd.
                  # Work is sliced KS chunks at a time into separate tiles so
                  # the per-chunk matmuls depend on their slice, not on one
                  # superchunk-wide op (finer overlap).
                  walls = []
                  for sl in range(KSUP // KS):
                      o = sl * KS
                      e8 = wpool.tile([P, KS * ws], BF16, tag=f"e8_{sl}")
                      nc.vector.tensor_tensor(
                          out=e8[:], in0=ssb[:, o * ws:(o + KS) * ws],
                          in1=adb[:, o * ws:(o + KS) * ws], op=ALU.add)
                      nc.scalar.activation(e8[:], e8[:], AF.Prelu, alpha=NEG)
                      wall = wpool.tile([P, KS * R], BF16, tag=f"wall_{sl}")
                      wall_v = wall[:].rearrange("p (k r) -> p k r", k=KS)
                      pz = wpool.tile([P, KS * ws], BF16, tag=f"pz_{sl}")
                      nc.scalar.activation(
                          pz[:].rearrange("p (k s) -> p k s", k=KS),
                          e8[:].rearrange("p (k s) -> p k s", k=KS), AF.Exp)
                      nc.vector.tensor_copy(
                          out=wall_v[:, :, wh:R],
                          in_=pz[:].rearrange("p (k s) -> p k s", k=KS))
                      if layer == 1:
                          # weight messages: gh * p via a middle-dim
                          # broadcast ((d,h) column order keeps the
                          # innermost read contiguous)
                          nc.vector.tensor_tensor(
                              out=wall_v[:, :, 0:wh].rearrange(
                                  "p k (d h) -> p k d h", d=dd),
                              in0=gh[:, o * wh:(o + KS) * wh].rearrange(
                                  "p (k d h) -> p k d h", k=KS, d=dd),
                              in1=pz[:].rearrange("p (k s) -> p k s", k=KS)
                              [:, :, None, :].to_broadcast([P, KS, dd, nh]),
                              op=ALU.mult)
                      else:
                          pw = wpool.tile([P, KS * wh], BF16, tag=f"pw_{sl}")
                          nc.scalar.activation(
                              pw[:].rearrange("p (k h d) -> p k h d",
                                              k=KS, h=nh),
                              e8[:].rearrange("p (k h) -> p k h", k=KS)
                              [:, :, :, None].to_broadcast([P, KS, nh, dd]),
                              AF.Exp)
                          nc.vector.tensor_tensor(
                              out=wall_v[:, :, 0:wh],
                              in0=gh[:, o * wh:(o + KS) * wh].rearrange(
                                  "p (k c) -> p k c", k=KS),
                              in1=pw[:].rearrange("p (k c) -> p k c", k=KS),
                              op=ALU.mult)
                      walls.append(wall)

                  for j in range(KSUP):
                      cidx = sc * KSUP + j
                      t = int(chunk_tile[cidx])
                      lhs = mtb[:, j * win:(j + 1) * win]
                      jj = j % KS
                      rhs = walls[j // KS][:, jj * R:(jj + 1) * R]
                      if t < 0:
                          nc.tensor.matmul(trash[0:win, :], lhs, rhs,
                                           start=not trash_used[0], stop=False,
                                           skip_group_check=True)
                          trash_used[0] = True
                          continue
                      b = int(chunk_b[cidx])
                      if chunk_start[cidx] and b == 0:
                          psum_by_tile[t] = ppool.tile([P, R], F32, tag="acc",
                                                       name=f"acc{t}")
                      ps = psum_by_tile[t]
                      nc.tensor.matmul(
                          ps[b:b + win, :], lhs, rhs,
                          start=bool(chunk_start[cidx]),
                          stop=bool(chunk_stop[cidx]),
                          skip_group_check=True)
                      te = int(chunk_epi[cidx])
                      if te >= 0:
                          epilogue(te, psum_by_tile.pop(te))
    nc.compile()
    return nc


# ---------------------------------------------------------------- orchestration

def make_A_inputs(inputs):
    x = np.asarray(inputs["x"], np.float32)
    W1 = np.ascontiguousarray(np.asarray(inputs["W1"], np.float32))
    a1s = np.asarray(inputs["a1_src"], np.float32)
    a1d = np.asarray(inputs["a1_dst"], np.float32)
    abd = np.zeros((HD, 16), np.float32)
    for h in range(H):
        abd[h * D:(h + 1) * D, h] = a1s[h]
        abd[h * D:(h + 1) * D, 8 + h] = a1d[h]
    in_maps = []
    for k in range(NCORES):
        lo = k * NSHARD
        xs = np.zeros((F_IN, NPAD_A), np.float32)
        xs[:, :NSHARD] = x[lo:lo + NSHARD].T
        xs = xs.reshape(4, P, NPAD_A).transpose(1, 0, 2).reshape(P, 4 * NPAD_A)
        xs = np.ascontiguousarray(xs).astype(ml_dtypes.bfloat16)
        in_maps.append({"XT": xs, "W1": W1, "ABD": abd})
    return in_maps


def tables_from_A(resA):
    ths, tss, tads = [], [], []
    for r in resA:
        tt = np.asarray(r["TT"])
        ths.append(np.ascontiguousarray(tt[0:HD, :NSHARD].T))
        tss.append(np.ascontiguousarray(tt[HD:HD + H, :NSHARD].T))
        tads.append(np.ascontiguousarray(tt[HD + H:80, :NSHARD].T))
    return np.concatenate(ths), np.concatenate(tss), np.concatenate(tads)


def make_B_inputs(g, ghs, sss, ads, inputs):
    return [{"GH": ghs[k], "SS": sss[k], "AD": ads[k], "MT": g.mtB[k],
             "B1": np.asarray(inputs["b1"], np.float32),
             "W2": np.ascontiguousarray(np.asarray(inputs["W2"], np.float32)),
             "A2S": np.asarray(inputs["a2_src"], np.float32).reshape(-1),
             "A2D": np.asarray(inputs["a2_dst"], np.float32).reshape(-1)}
            for k in range(NCORES)]


def make_C_inputs(g, ghs, sss, ads, inputs):
    return [{"GH": ghs[k], "SS": sss[k], "AD": ads[k], "MT": g.mtC[k],
             "B2": np.asarray(inputs["b2"], np.float32)}
            for k in range(NCORES)]


class GAT:
    def __init__(self, edge_index):
        self.metaB, self.placB, self.mtB = build_structure(edge_index, 64)
        # PSUM matmul outputs may only start at partition 0/32/64, so a
        # 32-node window layout (base 96) is not expressible; layer 2
        # shares the 64-window structure.
        self.metaC, self.placC, self.mtC = self.metaB, self.placB, self.mtB
        self.ncA = build_A()
        self.ncB = build_edge_launch_B(self.metaB)
        self.ncC = build_edge_launch(self.metaC, 2)

    def run(self, x, W1, a1_src, a1_dst, b1, W2, a2_src, a2_dst, b2,
            runner=run_bass_kernel_spmd):
        inputs = dict(x=x, W1=W1, a1_src=a1_src, a1_dst=a1_dst, b1=b1,
                      W2=W2, a2_src=a2_src, a2_dst=a2_dst, b2=b2)
        # ---- launch A
        in_maps = make_A_inputs(inputs)
        resA = runner(self.ncA, in_maps, core_ids=list(range(NCORES))).results
        th, tsrc, tad = tables_from_A(resA)

        # ---- streams for B (host layout)
        ghs, sss, ads = make_streams(self.placB, self.metaB["kt"],
                                     th, tsrc, tad, HD, H)
        in_maps = make_B_inputs(self, ghs, sss, ads, inputs)
        resB = runner(self.ncB, in_maps, core_ids=list(range(NCORES))).results
        t2 = np.concatenate([np.asarray(r["T2"])[:NSHARD] for r in resB])
        t2h, t2s, t2ad = t2[:, 0:C], t2[:, C:C + 1], t2[:, C + 1:C + 2]

        # ---- streams for C
        ghs, sss, ads = make_streams(self.placC, self.metaC["kt"],
                                     t2h, t2s, t2ad, C, 1)
        in_maps = make_C_inputs(self, ghs, sss, ads, inputs)
        resC = runner(self.ncC, in_maps, core_ids=list(range(NCORES))).results
        return np.concatenate(
            [np.asarray(r["OUT"])[:NSHARD] for r in resC]).astype(np.float32)


def kernel(x, edge_index, W1, a1_src, a1_dst, b1, W2, a2_src, a2_dst, b2):
    g = GAT(np.asarray(edge_index))
    return g.run(np.asarray(x, np.float32), np.asarray(W1), np.asarray(a1_src),
                 np.asarray(a1_dst), np.asarray(b1), np.asarray(W2),
                 np.asarray(a2_src), np.asarray(a2_dst), np.asarray(b2))
